# revision 1
# baseline (speedup 1.0000x reference)
"""Self-contained Trainium2 (Bass/Tile) DeformConv2d kernel.

kernel(x, offset, weight) -> np.ndarray [B, Cout, H, W] float32.
Data-parallel over batch: one SPMD Bass program per NeuronCore (8 cores).
Per core: bf16 x^T gather table in DRAM; DVE prep computes bilinear weights
(L128 layout) and pair-row gather indices (16-wrap layout, int16);
SWDGE dma_gather fetches 2-pixel channel rows; per-partition-scalar
multiplies + PE transpose-accumulate build val[c, j] in PSUM; per-tap
bf16 GEMM accumulates out[o, j] in PSUM over all 9 taps.
"""
import sys
import numpy as np

for _p in ("/opt/trn_rl_repo",):
    if _p not in sys.path:
        sys.path.insert(0, _p)

import concourse.bass as bass
import concourse.mybir as mybir
import concourse.tile as tile
from concourse import bacc
from concourse.masks import make_identity
from concourse.bass_utils import run_bass_kernel_spmd



f32 = mybir.dt.float32
bf16 = mybir.dt.bfloat16
i32 = mybir.dt.int32
i16 = mybir.dt.int16
Alu = mybir.AluOpType
P = 128


def build_dcn(C=256, Cout=256, H=64, W=64, KH=3, KW=3, PAD=1, CHUNK_JT=8,
              debug_prep=False, cast_round=True):
    HW = H * W
    S = HW // P
    SW = HW // 16
    NT = KH * KW
    CB = C // P
    MB = Cout // P
    assert S % CHUNK_JT == 0
    n_chunks = S // CHUNK_JT
    JC = CHUNK_JT * P
    NNB = (JC + 511) // 512
    FBIAS = 4.0 * max(H, W)
    # HW f32->int cast is round-nearest-even; CoreSim models truncation.
    FADD = FBIAS - (0.5 if cast_round else 0.0)

    nc = bacc.Bacc("TRN2", target_bir_lowering=False, debug=False)

    xt = nc.declare_dram_parameter("xt", [HW, C], f32, isOutput=False)
    offy = nc.declare_dram_parameter("offy", [P, NT, S], f32, isOutput=False)
    offx = nc.declare_dram_parameter("offx", [P, NT, S], f32, isOutput=False)
    byc = nc.declare_dram_parameter("byc", [P, NT, S], f32, isOutput=False)
    bxc = nc.declare_dram_parameter("bxc", [P, NT, S], f32, isOutput=False)
    offyW = nc.declare_dram_parameter("offyW", [P, NT, SW], f32, isOutput=False)
    offxW = nc.declare_dram_parameter("offxW", [P, NT, SW], f32, isOutput=False)
    bycW = nc.declare_dram_parameter("bycW", [P, NT, SW], f32, isOutput=False)
    bxcW = nc.declare_dram_parameter("bxcW", [P, NT, SW], f32, isOutput=False)
    wt = nc.declare_dram_parameter("wt", [P, NT, CB, Cout], f32, isOutput=False)
    out = nc.declare_dram_parameter("out", [Cout, HW], f32, isOutput=True)
    if debug_prep:
        dbg_w = nc.declare_dram_parameter("dbg_w", [4, P, NT, S], f32, isOutput=True)
        dbg_iA = nc.declare_dram_parameter("dbg_iA", [P, NT, SW], i32, isOutput=True)
        dbg_iB = nc.declare_dram_parameter("dbg_iB", [P, NT, SW], i32, isOutput=True)
        dbg_g = nc.declare_dram_parameter("dbg_g", [P, CHUNK_JT, 2 * C], f32, isOutput=True)
        dbg_v = nc.declare_dram_parameter("dbg_v", [P, CB, CHUNK_JT * P], f32, isOutput=True)

    xtb = nc.dram_tensor("xtb", [HW + 2, C], bf16)

    with tile.TileContext(nc) as tc:
        with tc.tile_pool(name="persist", bufs=1) as pe_pool:
            # persistent tiles
            wtb = pe_pool.tile([P, NT, CB, Cout], bf16, name="wtb")
            ident = pe_pool.tile([P, P], bf16, name="ident")
            w00 = pe_pool.tile([P, NT, S], f32, name="w00")
            w01 = pe_pool.tile([P, NT, S], f32, name="w01")
            w10 = pe_pool.tile([P, NT, S], f32, name="w10")
            w11 = pe_pool.tile([P, NT, S], f32, name="w11")
            idxA16 = pe_pool.tile([P, NT, SW], i16, name="idxA16")
            idxB16 = pe_pool.tile([P, NT, SW], i16, name="idxB16")

            make_identity(nc, ident[:])

            # ---- phase 0: xtb build + weight load (transient scratch)
            with tc.tile_pool(name="ph0", bufs=1) as s0:
                R = HW // P
                xt_sb = s0.tile([P, R * C], f32, name="xt_sb")
                nc.sync.dma_start(
                    out=xt_sb[:], in_=xt[:].rearrange("(p r) c -> p (r c)", p=P)
                )
                xt_bf = s0.tile([P, R * C], bf16, name="xt_bf")
                nc.vector.tensor_copy(out=xt_bf[:], in_=xt_sb[:])
                nc.sync.dma_start(
                    out=xtb[: HW].rearrange("(p r) c -> p (r c)", p=P), in_=xt_bf[:]
                )
                zpad = s0.tile([2, C], bf16, name="zpad")
                nc.vector.memset(zpad[:], 0.0)
                nc.sync.dma_start(out=xtb[HW : HW + 2], in_=zpad[:])
                wt_sb = s0.tile([P, NT * CB * Cout], f32, name="wt_sb")
                nc.sync.dma_start(
                    out=wt_sb[:], in_=wt[:].rearrange("p k b o -> p (k b o)")
                )
                nc.vector.tensor_copy(
                    out=wtb[:].rearrange("p k b o -> p (k b o)"), in_=wt_sb[:]
                )

            # ---- phase 1: L128 chain -> bilinear weights
            with tc.tile_pool(name="ph1", bufs=1) as sp:
                _ctr = [0]

                def newt(nm=None):
                    if nm is None:
                        _ctr[0] += 1
                        nm = f"pt{_ctr[0]}"
                    return sp.tile([P, NT, S], f32, name=nm)

                oy = newt("oy"); nc.sync.dma_start(out=oy[:], in_=offy[:])
                ox = newt("ox"); nc.sync.dma_start(out=ox[:], in_=offx[:])
                by = newt("by"); nc.sync.dma_start(out=by[:], in_=byc[:])
                bx = newt("bx"); nc.sync.dma_start(out=bx[:], in_=bxc[:])

                def tt(a, b, op):
                    o = newt()
                    nc.vector.tensor_tensor(out=o[:], in0=a[:], in1=b[:], op=op)
                    return o

                def ts(a, s1, op0, s2=None, op1=None):
                    o = newt()
                    if s2 is None:
                        nc.vector.tensor_scalar(
                            out=o[:], in0=a[:], scalar1=float(s1), scalar2=None,
                            op0=op0,
                        )
                    else:
                        nc.vector.tensor_scalar(
                            out=o[:], in0=a[:], scalar1=float(s1), scalar2=float(s2),
                            op0=op0, op1=op1,
                        )
                    return o

                py = tt(oy, by, Alu.add)
                px = tt(ox, bx, Alu.add)

                def floor_(v):
                    vb = ts(v, FADD, Alu.add)
                    vi = sp.tile([P, NT, S], i32, name=f"vi{_ctr[0]}")
                    nc.vector.tensor_copy(out=vi[:], in_=vb[:])
                    vf = newt()
                    nc.vector.tensor_copy(out=vf[:], in_=vi[:])
                    return ts(vf, FBIAS, Alu.subtract)

                y0 = floor_(py)
                x0 = floor_(px)
                ly = tt(py, y0, Alu.subtract)
                lx = tt(px, x0, Alu.subtract)

                def rng_mask(v, lo, hi):
                    a = ts(v, lo, Alu.is_ge)
                    b = ts(v, hi, Alu.is_le)
                    return tt(a, b, Alu.mult)

                my0 = rng_mask(y0, 0.0, H - 1)
                my1 = rng_mask(y0, -1.0, H - 2)
                mx0 = rng_mask(x0, 0.0, W - 1)
                mx1 = rng_mask(x0, -1.0, W - 2)

                one_m_ly = ts(ly, -1.0, Alu.mult, 1.0, Alu.add)
                one_m_lx = ts(lx, -1.0, Alu.mult, 1.0, Alu.add)
                vy0 = tt(one_m_ly, my0, Alu.mult)
                vy1 = tt(ly, my1, Alu.mult)
                ax0 = tt(one_m_lx, mx0, Alu.mult)
                ax1 = tt(lx, mx1, Alu.mult)

                sx = ts(x0, 0.0, Alu.max, W - 2, Alu.min)
                tsh = tt(x0, sx, Alu.subtract)
                e0 = ts(tsh, 0.0, Alu.is_equal)
                em1 = ts(tsh, -1.0, Alu.is_equal)
                e1 = ts(tsh, 1.0, Alu.is_equal)

                u0 = tt(tt(ax0, e0, Alu.mult), tt(ax1, em1, Alu.mult), Alu.add)
                u1 = tt(tt(ax0, e1, Alu.mult), tt(ax1, e0, Alu.mult), Alu.add)

                nc.vector.tensor_tensor(out=w00[:], in0=vy0[:], in1=u0[:], op=Alu.mult)
                nc.vector.tensor_tensor(out=w01[:], in0=vy0[:], in1=u1[:], op=Alu.mult)
                nc.vector.tensor_tensor(out=w10[:], in0=vy1[:], in1=u0[:], op=Alu.mult)
                nc.vector.tensor_tensor(out=w11[:], in0=vy1[:], in1=u1[:], op=Alu.mult)

            # ---- phase 2: W16 chain -> gather indices (per-tap to bound SBUF)
            with tc.tile_pool(name="ph2", bufs=1) as sq:
                _c2 = [0]

                def newq(nm=None):
                    if nm is None:
                        _c2[0] += 1
                        nm = f"qt{_c2[0]}"
                    return sq.tile([P, SW], f32, name=nm)

                def qtt(a, b, op, o=None):
                    o = o or newq()
                    nc.vector.tensor_tensor(out=o[:], in0=a[:], in1=b[:], op=op)
                    return o

                def qts(a, s1, op0, s2=None, op1=None, o=None):
                    o = o or newq()
                    if s2 is None:
                        nc.vector.tensor_scalar(
                            out=o[:], in0=a[:], scalar1=float(s1), scalar2=None,
                            op0=op0,
                        )
                    else:
                        nc.vector.tensor_scalar(
                            out=o[:], in0=a[:], scalar1=float(s1), scalar2=float(s2),
                            op0=op0, op1=op1,
                        )
                    return o

                oyq = newq("oyq"); oxq = newq("oxq")
                byq = newq("byq"); bxq = newq("bxq")
                pyq = newq("pyq"); pxq = newq("pxq")
                viq = sq.tile([P, SW], i32, name="viq")

                def qfloor(v, o=None):
                    vb = qts(v, FADD, Alu.add)
                    nc.vector.tensor_copy(out=viq[:], in_=vb[:])
                    nc.vector.tensor_copy(out=vb[:], in_=viq[:])
                    return qts(vb, FBIAS, Alu.subtract, o=o)

                y0q = newq("y0q"); x0q = newq("x0q")
                sxq = newq("sxq"); yc0q = newq("yc0q"); yc1q = newq("yc1q")
                iAf = newq("iAf"); iBf = newq("iBf")

                for k in range(NT):
                    nc.sync.dma_start(out=oyq[:], in_=offyW[:, k, :])
                    nc.sync.dma_start(out=oxq[:], in_=offxW[:, k, :])
                    nc.sync.dma_start(out=byq[:], in_=bycW[:, k, :])
                    nc.sync.dma_start(out=bxq[:], in_=bxcW[:, k, :])
                    qtt(oyq, byq, Alu.add, o=pyq)
                    qtt(oxq, bxq, Alu.add, o=pxq)
                    qfloor(pyq, o=y0q)
                    qfloor(pxq, o=x0q)
                    qts(x0q, 0.0, Alu.max, W - 2, Alu.min, o=sxq)
                    qts(y0q, 0.0, Alu.max, H - 1, Alu.min, o=yc0q)
                    t1 = qts(y0q, 1.0, Alu.add, 0.0, Alu.max)
                    qts(t1, H - 1, Alu.min, o=yc1q)
                    nc.vector.scalar_tensor_tensor(
                        out=iAf[:], in0=yc0q[:], scalar=float(W), in1=sxq[:],
                        op0=Alu.mult, op1=Alu.add,
                    )
                    nc.vector.scalar_tensor_tensor(
                        out=iBf[:], in0=yc1q[:], scalar=float(W), in1=sxq[:],
                        op0=Alu.mult, op1=Alu.add,
                    )
                    nc.vector.tensor_copy(out=idxA16[:, k, :], in_=iAf[:])
                    nc.vector.tensor_copy(out=idxB16[:, k, :], in_=iBf[:])

            if debug_prep:
                with tc.tile_pool(name="dbg", bufs=1) as dpool:
                    for wi, wv in enumerate((w00, w01, w10, w11)):
                        nc.sync.dma_start(out=dbg_w[wi], in_=wv[:])
                    dA = dpool.tile([P, NT, SW], i32, name="dA")
                    nc.vector.tensor_copy(out=dA[:], in_=idxA16[:])
                    nc.sync.dma_start(out=dbg_iA[:], in_=dA[:])
                    dB = dpool.tile([P, NT, SW], i32, name="dB")
                    nc.vector.tensor_copy(out=dB[:], in_=idxB16[:])
                    nc.sync.dma_start(out=dbg_iB[:], in_=dB[:])

            # overlapping-pair view of the bf16 table: row i covers elements
            # [C*i, C*i + 2C) — dma_gather elem_step=C, elem_size=2C.
            xtb_pairs = bass.AP(xtb[:].tensor, 0, [[C, HW], [1, 2 * C]])

            # ---- main loop
            with (
                tc.tile_pool(name="gather", bufs=2) as g_pool,
                tc.tile_pool(name="prod", bufs=2) as pr_pool,
                tc.tile_pool(name="vout", bufs=2) as v_pool,
                tc.tile_pool(name="obuf", bufs=2) as o_pool,
                tc.tile_pool(name="psum_out", bufs=1, space="PSUM") as pso_pool,
                tc.tile_pool(name="psum_val", bufs=1, space="PSUM") as psv_pool,
            ):
                SWC = JC // 16  # idx slots per chunk
                for ch in range(n_chunks):
                    out_ps = [
                        pso_pool.tile([P, JC], f32, space="PSUM", name=f"out_ps{_m}")
                        for _m in range(MB)
                    ]
                    for k in range(NT):
                        gA = g_pool.tile([P, CHUNK_JT, 2 * C], bf16, name="gA")
                        gB = g_pool.tile([P, CHUNK_JT, 2 * C], bf16, name="gB")
                        isl = slice(ch * SWC, (ch + 1) * SWC)
                        nc.gpsimd.dma_gather(
                            gA[:], xtb_pairs, idxA16[:, k, isl], JC, JC, 2 * C,
                            elem_step=C,
                        )
                        nc.gpsimd.dma_gather(
                            gB[:], xtb_pairs, idxB16[:, k, isl], JC, JC, 2 * C,
                            elem_step=C,
                        )
                        if debug_prep and ch == 0 and k == 0:
                            dg = pr_pool.tile([P, CHUNK_JT, 2 * C], f32, name="dg")
                            nc.vector.tensor_copy(out=dg[:], in_=gA[:])
                            nc.sync.dma_start(out=dbg_g[:], in_=dg[:])
                        val_ps = [
                            psv_pool.tile([P, JC], f32, space="PSUM",
                                          name=f"val_ps{_c}")
                            for _c in range(CB)
                        ]
                        for jt in range(CHUNK_JT):
                            s_idx = ch * CHUNK_JT + jt
                            pr = pr_pool.tile([P, 4, C], bf16, name="pr")
                            pieces = [
                                (gA, slice(0, C), w00),
                                (gA, slice(C, 2 * C), w01),
                                (gB, slice(0, C), w10),
                                (gB, slice(C, 2 * C), w11),
                            ]
                            for n, (g, csl, wv) in enumerate(pieces):
                                nc.vector.tensor_scalar(
                                    out=pr[:, n, :], in0=g[:, jt, csl],
                                    scalar1=wv[:, k, s_idx : s_idx + 1],
                                    scalar2=None, op0=Alu.mult,
                                )
                            for n in range(4):
                                for cb in range(CB):
                                    nc.tensor.matmul(
                                        out=val_ps[cb][:, jt * P : (jt + 1) * P],
                                        lhsT=pr[:, n, cb * P : (cb + 1) * P],
                                        rhs=ident[:],
                                        start=(n == 0),
                                        stop=(n == 3),
                                    )
                        vsb = v_pool.tile([P, CB, JC], bf16, name="vsb")
                        for cb in range(CB):
                            nc.vector.tensor_copy(out=vsb[:, cb, :], in_=val_ps[cb][:])
                        if debug_prep and ch == 0 and k == 0:
                            dv = v_pool.tile([P, CB, JC], f32, name="dv")
                            nc.vector.tensor_copy(out=dv[:], in_=vsb[:])
                            nc.sync.dma_start(out=dbg_v[:], in_=dv[:])
                        for mb in range(MB):
                            for cb in range(CB):
                                for nb in range(NNB):
                                    nsl = slice(nb * 512, min((nb + 1) * 512, JC))
                                    nc.tensor.matmul(
                                        out=out_ps[mb][:, nsl],
                                        lhsT=wtb[:, k, cb, mb * P : (mb + 1) * P],
                                        rhs=vsb[:, cb, nsl],
                                        start=(k == 0 and cb == 0),
                                        stop=(k == NT - 1 and cb == CB - 1),
                                    )
                    for mb in range(MB):
                        ob = o_pool.tile([P, JC], f32, name="ob")
                        nc.vector.tensor_copy(out=ob[:], in_=out_ps[mb][:])
                        nc.sync.dma_start(
                            out=out[mb * P : (mb + 1) * P, ch * JC : (ch + 1) * JC],
                            in_=ob[:],
                        )

    nc.compile()
    return nc


def host_prep(x_b, offset_b, weight, H, W, KH, KW, PAD):
    """Per-core input map from one batch slice (numpy, f32)."""
    C = x_b.shape[0]
    Cout = weight.shape[0]
    HW = H * W
    S = HW // P
    SW = HW // 16
    NT = KH * KW
    CB = C // P
    xt = np.ascontiguousarray(x_b.reshape(C, HW).T).astype(np.float32)
    off = offset_b.reshape(NT, 2, HW)
    j = np.arange(HW)
    ks = np.arange(NT)
    byv = (j[None, :] // W - PAD + (ks // KW)[:, None]).astype(np.float32)  # [k, j]
    bxv = (j[None, :] % W - PAD + (ks % KW)[:, None]).astype(np.float32)

    def l128(a):  # [k, j] -> [p, k, s], j = 128*s + p
        return np.ascontiguousarray(a.reshape(NT, S, P).transpose(2, 0, 1)).astype(np.float32)

    def w16(a):  # [k, j] -> [q + 16g, k, s], j = 16*s + q, replicated over g
        b = a.reshape(NT, SW, 16).transpose(2, 0, 1)  # [q, k, s]
        return np.ascontiguousarray(np.tile(b, (8, 1, 1))).astype(np.float32)

    wr = weight.reshape(Cout, C, NT)
    wtv = wr.reshape(Cout, CB, P, NT).transpose(2, 3, 1, 0)
    return {
        "xt": xt,
        "offy": l128(off[:, 0]), "offx": l128(off[:, 1]),
        "byc": l128(byv), "bxc": l128(bxv),
        "offyW": w16(off[:, 0]), "offxW": w16(off[:, 1]),
        "bycW": w16(byv), "bxcW": w16(bxv),
        "wt": np.ascontiguousarray(wtv).astype(np.float32),
    }


_NC_CACHE = {}


def _get_nc(key, **kw):
    if key not in _NC_CACHE:
        _NC_CACHE[key] = build_dcn(**kw)
    return _NC_CACHE[key]


def kernel(x, offset, weight):
    x = np.asarray(x, dtype=np.float32)
    offset = np.asarray(offset, dtype=np.float32)
    weight = np.asarray(weight, dtype=np.float32)
    B, C, H, W = x.shape
    Cout = weight.shape[0]
    KH, KW = weight.shape[2], weight.shape[3]
    PAD = 1
    assert B == 8 and C % 128 == 0 and Cout % 128 == 0
    nc = _get_nc((C, Cout, H, W, KH, KW), C=C, Cout=Cout, H=H, W=W,
                 KH=KH, KW=KW, PAD=PAD, CHUNK_JT=8)
    in_maps = [host_prep(x[b], offset[b], weight, H, W, KH, KW, PAD)
               for b in range(B)]
    res = run_bass_kernel_spmd(nc, in_maps, list(range(B)))
    out = np.stack([res.results[b]["out"].reshape(Cout, H, W) for b in range(B)])
    return out.astype(np.float32)



# revision 4
# speedup vs baseline: 1.3127x; 1.3127x over previous
"""Self-contained Trainium2 (Bass/Tile) DeformConv2d kernel.

kernel(x, offset, weight) -> np.ndarray [B, Cout, H, W] float32.
Data-parallel over batch: one SPMD Bass program per NeuronCore (8 cores).

v2 design (vs v1 baseline):
- All bilinear weights / gather indices / gather table are built on the host
  (numpy) and passed as DRAM inputs; no on-device prep phases.
- Gather table is a bf16 "pair-row" layout: entry (y, x) holds image rows y
  and y+1 at column x concatenated (2C values). One 4C-value descriptor per
  sample covers all 4 bilinear neighbors (half the gather instructions and
  descriptor-gen of v1; DMA bytes unchanged).
- DVE does only the 4 per-sample weight multiplies (tensor_scalar, 4x mode).
- PE transpose-accumulates the 4 pieces into channel-major val (PSUM), then
  runs the per-tap GEMM.
- All PSUM->SBUF copies run on the otherwise-idle Activation engine.
- Output is written bf16 and upcast on the host.
"""
import sys
import numpy as np
import ml_dtypes

for _p in ("/opt/trn_rl_repo",):
    if _p not in sys.path:
        sys.path.insert(0, _p)

import concourse.bass as bass
import concourse.mybir as mybir
import concourse.tile as tile
from concourse import bacc
from concourse.masks import make_identity
from concourse.bass_utils import run_bass_kernel_spmd

f32 = mybir.dt.float32
bf16 = mybir.dt.bfloat16
i16 = mybir.dt.int16
Alu = mybir.AluOpType
P = 128
BF16 = ml_dtypes.bfloat16


def build_dcn(C=256, Cout=256, H=64, W=64, KH=3, KW=3, CHUNK_JT=8):
    HW = H * W
    S = HW // P              # 32 pixel slots of 128
    NT = KH * KW             # 9 taps
    CB = C // P              # 2 input-channel blocks
    MB = Cout // P           # 2 output-channel blocks
    assert S % CHUNK_JT == 0
    n_chunks = S // CHUNK_JT
    JC = CHUNK_JT * P        # 1024 pixels per chunk
    SWC = JC // 16           # idx columns per chunk (16-wrap layout)
    NNB = JC // 512          # moving-dim blocks for the GEMM
    TROWS = (H + 1) * W      # pair-table rows

    nc = bacc.Bacc("TRN2", target_bir_lowering=False, debug=False)

    # one extra zero row backs the overlapping pair view's last entry
    tbl = nc.declare_dram_parameter("tbl", [TROWS + 1, 2 * C], bf16,
                                    isOutput=False)
    idx = nc.declare_dram_parameter("idx", [P, NT, n_chunks * SWC], i16,
                                    isOutput=False)
    w4 = nc.declare_dram_parameter("w4", [P, NT, S, 4], f32, isOutput=False)
    wt = nc.declare_dram_parameter("wt", [P, NT, CB, Cout], bf16, isOutput=False)
    out = nc.declare_dram_parameter("out", [Cout, HW], bf16, isOutput=True)

    with tile.TileContext(nc) as tc:
        with tc.tile_pool(name="persist", bufs=1) as pp:
            wtb = pp.tile([P, NT, CB, Cout], bf16, name="wtb")
            w4t = pp.tile([P, NT, S, 4], f32, name="w4t")
            idxt = pp.tile([P, NT, n_chunks * SWC], i16, name="idxt")
            ident = pp.tile([P, P], bf16, name="ident")

            nc.sync.dma_start(out=wtb[:], in_=wt[:])
            nc.sync.dma_start(out=w4t[:], in_=w4[:])
            nc.sync.dma_start(out=idxt[:], in_=idx[:])
            make_identity(nc, ident[:])

            # overlapping-pair view: entry i covers table elements
            # [2C*i, 2C*i + 4C) -> one descriptor = 4 bilinear neighbors.
            tbl_pairs = bass.AP(tbl[:].tensor, 0, [[2 * C, TROWS], [1, 4 * C]])

            with (
                tc.tile_pool(name="gather", bufs=2) as g_pool,
                tc.tile_pool(name="prod", bufs=2) as pr_pool,
                tc.tile_pool(name="vout", bufs=2) as v_pool,
                tc.tile_pool(name="obuf", bufs=2) as o_pool,
                tc.tile_pool(name="psum_out", bufs=1, space="PSUM") as pso_pool,
                tc.tile_pool(name="psum_val", bufs=1, space="PSUM") as psv_pool,
            ):
                for ch in range(n_chunks):
                    out_ps = [
                        pso_pool.tile([P, JC], f32, space="PSUM", name=f"out_ps{m}")
                        for m in range(MB)
                    ]
                    for k in range(NT):
                        g = g_pool.tile([P, CHUNK_JT, 4 * C], bf16, name="g")
                        nc.gpsimd.dma_gather(
                            g[:], tbl_pairs,
                            idxt[:, k, ch * SWC:(ch + 1) * SWC],
                            JC, JC, 4 * C, elem_step=2 * C,
                        )
                        val_ps = [
                            psv_pool.tile([P, JC], f32, space="PSUM",
                                          name=f"val_ps{c}")
                            for c in range(CB)
                        ]
                        for jt in range(CHUNK_JT):
                            s_idx = ch * CHUNK_JT + jt
                            pr = pr_pool.tile([P, 4, C], bf16, name="pr")
                            for n in range(4):
                                nc.vector.tensor_scalar(
                                    out=pr[:, n, :],
                                    in0=g[:, jt, n * C:(n + 1) * C],
                                    scalar1=w4t[:, k, s_idx, n:n + 1],
                                    scalar2=None, op0=Alu.mult,
                                )
                            for n in range(4):
                                for cb in range(CB):
                                    nc.tensor.matmul(
                                        out=val_ps[cb][:, jt * P:(jt + 1) * P],
                                        lhsT=pr[:, n, cb * P:(cb + 1) * P],
                                        rhs=ident[:],
                                        start=(n == 0), stop=(n == 3),
                                    )
                        vsb = v_pool.tile([P, CB, JC], bf16, name="vsb")
                        for cb in range(CB):
                            nc.scalar.copy(out=vsb[:, cb, :], in_=val_ps[cb][:])
                        for mb in range(MB):
                            for cb in range(CB):
                                for nb in range(NNB):
                                    nsl = slice(nb * 512, (nb + 1) * 512)
                                    nc.tensor.matmul(
                                        out=out_ps[mb][:, nsl],
                                        lhsT=wtb[:, k, cb, mb * P:(mb + 1) * P],
                                        rhs=vsb[:, cb, nsl],
                                        start=(k == 0 and cb == 0),
                                        stop=(k == NT - 1 and cb == CB - 1),
                                    )
                    for mb in range(MB):
                        ob = o_pool.tile([P, JC], bf16, name="ob")
                        nc.scalar.copy(out=ob[:], in_=out_ps[mb][:])
                        nc.sync.dma_start(
                            out=out[mb * P:(mb + 1) * P, ch * JC:(ch + 1) * JC],
                            in_=ob[:],
                        )

    nc.compile()
    return nc


def host_prep(x_b, offset_b, weight, H, W, KH, KW, PAD):
    """Per-core input map from one batch slice (numpy, f32)."""
    C = x_b.shape[0]
    Cout = weight.shape[0]
    HW = H * W
    S = HW // P
    NT = KH * KW
    CB = C // P

    # pair-row gather table: entry r=(y0+1)*W+x holds rows (y0, y0+1) at col x
    xt = x_b.reshape(C, H, W).transpose(1, 2, 0).astype(np.float32)  # [H, W, C]
    Z = np.zeros((H + 2, W, C), np.float32)
    Z[1:H + 1] = xt
    T = np.concatenate([Z[0:H + 1], Z[1:H + 2]], axis=-1)  # [(H+1), W, 2C]
    tbl = np.zeros(((H + 1) * W + 1, 2 * C), np.float32)
    tbl[:-1] = T.reshape((H + 1) * W, 2 * C)
    tbl = tbl.astype(BF16)

    # sample coords per (tap, pixel)
    off = offset_b.reshape(NT, 2, HW).astype(np.float32)
    j = np.arange(HW)
    ks = np.arange(NT)
    by = (j[None, :] // W - PAD + (ks // KW)[:, None]).astype(np.float32)
    bx = (j[None, :] % W - PAD + (ks % KW)[:, None]).astype(np.float32)
    py = by + off[:, 0]
    px = bx + off[:, 1]
    y0 = np.floor(py)
    x0 = np.floor(px)
    ly = (py - y0).astype(np.float32)
    lx = (px - x0).astype(np.float32)
    qy = np.clip(y0, -1, H - 1)
    sx = np.clip(x0, 0, W - 2)
    idx_lin = ((qy + 1) * W + sx).astype(np.int16)  # [NT, HW]

    wy0 = (1.0 - ly) * ((y0 >= 0) & (y0 <= H - 1))
    wyB = ly * ((y0 >= -1) & (y0 <= H - 2))
    vx0 = (x0 >= 0) & (x0 <= W - 1)
    vx1 = (x0 >= -1) & (x0 <= W - 2)
    wxA = (1.0 - lx) * vx0 * (x0 == sx) + lx * vx1 * ((x0 + 1) == sx)
    wxB = (1.0 - lx) * vx0 * (x0 == (sx + 1)) + lx * vx1 * ((x0 + 1) == (sx + 1))
    # piece order matches the gathered 4C row: [y0|x0, y1|x0, y0|x1, y1|x1]
    w4 = np.stack([wy0 * wxA, wyB * wxA, wy0 * wxB, wyB * wxB],
                  axis=-1).astype(np.float32)  # [NT, HW, 4]
    w4d = np.ascontiguousarray(
        w4.reshape(NT, S, P, 4).transpose(2, 0, 1, 3))  # [P, NT, S, 4]

    # 16-wrap idx layout: slice column c of chunk ch, partition q -> sample
    # i = c*16 + q (i = chunk-local pixel), replicated over 8 partition groups
    idxw = idx_lin.reshape(NT, HW // 16, 16).transpose(2, 0, 1)  # [16, NT, HW/16]
    idxw = np.ascontiguousarray(np.tile(idxw, (8, 1, 1))).astype(np.int16)

    wtd = np.ascontiguousarray(
        weight.reshape(Cout, CB, P, NT).transpose(2, 3, 1, 0)).astype(BF16)
    return {"tbl": tbl, "idx": idxw, "w4": w4d, "wt": wtd}


_NC_CACHE = {}


def _get_nc(key, **kw):
    if key not in _NC_CACHE:
        _NC_CACHE[key] = build_dcn(**kw)
    return _NC_CACHE[key]


def kernel(x, offset, weight):
    x = np.asarray(x, dtype=np.float32)
    offset = np.asarray(offset, dtype=np.float32)
    weight = np.asarray(weight, dtype=np.float32)
    B, C, H, W = x.shape
    Cout = weight.shape[0]
    KH, KW = weight.shape[2], weight.shape[3]
    assert B == 8 and C % 128 == 0 and Cout % 128 == 0
    nc = _get_nc((C, Cout, H, W, KH, KW), C=C, Cout=Cout, H=H, W=W,
                 KH=KH, KW=KW, CHUNK_JT=8)
    in_maps = [host_prep(x[b], offset[b], weight, H, W, KH, KW, 1)
               for b in range(B)]
    res = run_bass_kernel_spmd(nc, in_maps, list(range(B)))
    out = np.stack([
        np.asarray(res.results[b]["out"]).astype(np.float32).reshape(Cout, H, W)
        for b in range(B)
    ])
    return out


# revision 10
# speedup vs baseline: 1.3846x; 1.0548x over previous
"""Self-contained Trainium2 (Bass/Tile) DeformConv2d kernel.

kernel(x, offset, weight) -> np.ndarray [B, Cout, H, W] float32.
Data-parallel over batch: one SPMD Bass program per NeuronCore (8 cores).

v2 design (vs v1 baseline):
- All bilinear weights / gather indices / gather table are built on the host
  (numpy) and passed as DRAM inputs; no on-device prep phases.
- Gather table is a bf16 "pair-row" layout: entry (y, x) holds image rows y
  and y+1 at column x concatenated (2C values). One 4C-value descriptor per
  sample covers all 4 bilinear neighbors (half the gather instructions and
  descriptor-gen of v1; DMA bytes unchanged).
- DVE does only the 4 per-sample weight multiplies (tensor_scalar, 4x mode).
- PE transpose-accumulates the 4 pieces into channel-major val (PSUM), then
  runs the per-tap GEMM.
- All PSUM->SBUF copies run on the otherwise-idle Activation engine.
- Output is written bf16 and upcast on the host.
"""
import sys
import numpy as np
import ml_dtypes

for _p in ("/opt/trn_rl_repo",):
    if _p not in sys.path:
        sys.path.insert(0, _p)

import concourse.bass as bass
import concourse.mybir as mybir
import concourse.tile as tile
from concourse import bacc
from concourse.masks import make_identity
from concourse.bass_utils import run_bass_kernel_spmd

f32 = mybir.dt.float32
bf16 = mybir.dt.bfloat16
i16 = mybir.dt.int16
Alu = mybir.AluOpType
P = 128
BF16 = ml_dtypes.bfloat16


def build_dcn(C=256, Cout=256, H=64, W=64, KH=3, KW=3, CHUNK_JT=8):
    HW = H * W
    S = HW // P              # 32 pixel slots of 128
    NT = KH * KW             # 9 taps
    CB = C // P              # 2 input-channel blocks
    MB = Cout // P           # 2 output-channel blocks
    assert S % CHUNK_JT == 0
    n_chunks = S // CHUNK_JT
    JC = CHUNK_JT * P        # 1024 pixels per chunk
    SWC = JC // 16           # idx columns per chunk (16-wrap layout)
    NNB = JC // 512          # moving-dim blocks for the GEMM
    TROWS = (H + 1) * W      # pair-table rows

    nc = bacc.Bacc("TRN2", target_bir_lowering=False, debug=False)

    # one extra zero row backs the overlapping pair view's last entry
    tbl = nc.declare_dram_parameter("tbl", [TROWS + 1, 2 * C], bf16,
                                    isOutput=False)
    idx = nc.declare_dram_parameter("idx", [P, NT, n_chunks * SWC], i16,
                                    isOutput=False)
    w4 = nc.declare_dram_parameter("w4", [P, NT, S, 4], f32, isOutput=False)
    wt = nc.declare_dram_parameter("wt", [P, NT, CB, Cout], bf16, isOutput=False)
    out = nc.declare_dram_parameter("out", [Cout, HW], bf16, isOutput=True)

    with tile.TileContext(nc) as tc:
        with tc.tile_pool(name="persist", bufs=1) as pp:
            wtb = pp.tile([P, NT, CB, Cout], bf16, name="wtb")
            w4t = pp.tile([P, NT, S, 4], f32, name="w4t")
            idxt = pp.tile([P, NT, n_chunks * SWC], i16, name="idxt")
            ident = pp.tile([P, P], bf16, name="ident")

            nc.sync.dma_start(out=wtb[:], in_=wt[:])
            nc.sync.dma_start(out=w4t[:], in_=w4[:])
            nc.sync.dma_start(out=idxt[:], in_=idx[:])
            make_identity(nc, ident[:])

            # overlapping-pair view: entry i covers table elements
            # [2C*i, 2C*i + 4C) -> one descriptor = 4 bilinear neighbors.
            tbl_pairs = bass.AP(tbl[:].tensor, 0, [[2 * C, TROWS], [1, 4 * C]])

            with (
                tc.tile_pool(name="gather", bufs=2) as g_pool,
                tc.tile_pool(name="prod", bufs=2) as pr_pool,
                tc.tile_pool(name="vout", bufs=2) as v_pool,
                tc.tile_pool(name="obuf", bufs=2) as o_pool,
                tc.tile_pool(name="psum_out", bufs=1, space="PSUM") as pso_pool,
                tc.tile_pool(name="psum_val", bufs=2, space="PSUM") as psv_pool,
            ):
                for ch in range(n_chunks):
                    out_ps = [
                        pso_pool.tile([P, JC], f32, space="PSUM", name=f"out_ps{m}")
                        for m in range(MB)
                    ]
                    for k in range(NT):
                        g = g_pool.tile([P, CHUNK_JT, 4 * C], bf16, name="g")
                        nc.gpsimd.dma_gather(
                            g[:], tbl_pairs,
                            idxt[:, k, ch * SWC:(ch + 1) * SWC],
                            JC, JC, 4 * C, elem_step=2 * C,
                        )
                        pr = pr_pool.tile([P, CHUNK_JT, 4, C], bf16, name="pr")
                        for jt in range(CHUNK_JT):
                            s_idx = ch * CHUNK_JT + jt
                            for n in range(4):
                                nc.vector.tensor_scalar(
                                    out=pr[:, jt, n, :],
                                    in0=g[:, jt, n * C:(n + 1) * C],
                                    scalar1=w4t[:, k, s_idx, n:n + 1],
                                    scalar2=None, op0=Alu.mult,
                                )
                        vsb = v_pool.tile([P, CB, JC], bf16, name="vsb")
                        for cb in range(CB):
                            # one cb in flight at a time: 2 PSUM val tiles
                            # (pool bufs=2) so PE never waits on the Act copy
                            val_ps = psv_pool.tile([P, JC], f32, space="PSUM",
                                                   name="val_ps")
                            for jt in range(CHUNK_JT):
                                for n in range(4):
                                    nc.tensor.matmul(
                                        out=val_ps[:, jt * P:(jt + 1) * P],
                                        lhsT=pr[:, jt, n, cb * P:(cb + 1) * P],
                                        rhs=ident[:],
                                        start=(n == 0), stop=(n == 3),
                                    )
                            nc.scalar.copy(out=vsb[:, cb, :], in_=val_ps[:])
                            for mb in range(MB):
                                for nb in range(NNB):
                                    nsl = slice(nb * 512, (nb + 1) * 512)
                                    nc.tensor.matmul(
                                        out=out_ps[mb][:, nsl],
                                        lhsT=wtb[:, k, cb, mb * P:(mb + 1) * P],
                                        rhs=vsb[:, cb, nsl],
                                        start=(k == 0 and cb == 0),
                                        stop=(k == NT - 1 and cb == CB - 1),
                                    )
                    for mb in range(MB):
                        ob = o_pool.tile([P, JC], bf16, name="ob")
                        nc.scalar.copy(out=ob[:], in_=out_ps[mb][:])
                        nc.sync.dma_start(
                            out=out[mb * P:(mb + 1) * P, ch * JC:(ch + 1) * JC],
                            in_=ob[:],
                        )

    nc.compile()
    return nc


def host_prep(x_b, offset_b, weight, H, W, KH, KW, PAD):
    """Per-core input map from one batch slice (numpy, f32)."""
    C = x_b.shape[0]
    Cout = weight.shape[0]
    HW = H * W
    S = HW // P
    NT = KH * KW
    CB = C // P

    # pair-row gather table: entry r=(y0+1)*W+x holds rows (y0, y0+1) at col x
    xt = x_b.reshape(C, H, W).transpose(1, 2, 0).astype(np.float32)  # [H, W, C]
    Z = np.zeros((H + 2, W, C), np.float32)
    Z[1:H + 1] = xt
    T = np.concatenate([Z[0:H + 1], Z[1:H + 2]], axis=-1)  # [(H+1), W, 2C]
    tbl = np.zeros(((H + 1) * W + 1, 2 * C), np.float32)
    tbl[:-1] = T.reshape((H + 1) * W, 2 * C)
    tbl = tbl.astype(BF16)

    # sample coords per (tap, pixel)
    off = offset_b.reshape(NT, 2, HW).astype(np.float32)
    j = np.arange(HW)
    ks = np.arange(NT)
    by = (j[None, :] // W - PAD + (ks // KW)[:, None]).astype(np.float32)
    bx = (j[None, :] % W - PAD + (ks % KW)[:, None]).astype(np.float32)
    py = by + off[:, 0]
    px = bx + off[:, 1]
    y0 = np.floor(py)
    x0 = np.floor(px)
    ly = (py - y0).astype(np.float32)
    lx = (px - x0).astype(np.float32)
    qy = np.clip(y0, -1, H - 1)
    sx = np.clip(x0, 0, W - 2)
    idx_lin = ((qy + 1) * W + sx).astype(np.int16)  # [NT, HW]

    wy0 = (1.0 - ly) * ((y0 >= 0) & (y0 <= H - 1))
    wyB = ly * ((y0 >= -1) & (y0 <= H - 2))
    vx0 = (x0 >= 0) & (x0 <= W - 1)
    vx1 = (x0 >= -1) & (x0 <= W - 2)
    wxA = (1.0 - lx) * vx0 * (x0 == sx) + lx * vx1 * ((x0 + 1) == sx)
    wxB = (1.0 - lx) * vx0 * (x0 == (sx + 1)) + lx * vx1 * ((x0 + 1) == (sx + 1))
    # piece order matches the gathered 4C row: [y0|x0, y1|x0, y0|x1, y1|x1]
    w4 = np.stack([wy0 * wxA, wyB * wxA, wy0 * wxB, wyB * wxB],
                  axis=-1).astype(np.float32)  # [NT, HW, 4]
    w4d = np.ascontiguousarray(
        w4.reshape(NT, S, P, 4).transpose(2, 0, 1, 3))  # [P, NT, S, 4]

    # 16-wrap idx layout: slice column c of chunk ch, partition q -> sample
    # i = c*16 + q (i = chunk-local pixel), replicated over 8 partition groups
    idxw = idx_lin.reshape(NT, HW // 16, 16).transpose(2, 0, 1)  # [16, NT, HW/16]
    idxw = np.ascontiguousarray(np.tile(idxw, (8, 1, 1))).astype(np.int16)

    wtd = np.ascontiguousarray(
        weight.reshape(Cout, CB, P, NT).transpose(2, 3, 1, 0)).astype(BF16)
    return {"tbl": tbl, "idx": idxw, "w4": w4d, "wt": wtd}


_NC_CACHE = {}


def _get_nc(key, **kw):
    if key not in _NC_CACHE:
        _NC_CACHE[key] = build_dcn(**kw)
    return _NC_CACHE[key]


def kernel(x, offset, weight):
    x = np.asarray(x, dtype=np.float32)
    offset = np.asarray(offset, dtype=np.float32)
    weight = np.asarray(weight, dtype=np.float32)
    B, C, H, W = x.shape
    Cout = weight.shape[0]
    KH, KW = weight.shape[2], weight.shape[3]
    assert B == 8 and C % 128 == 0 and Cout % 128 == 0
    nc = _get_nc((C, Cout, H, W, KH, KW), C=C, Cout=Cout, H=H, W=W,
                 KH=KH, KW=KW, CHUNK_JT=8)
    in_maps = [host_prep(x[b], offset[b], weight, H, W, KH, KW, 1)
               for b in range(B)]
    res = run_bass_kernel_spmd(nc, in_maps, list(range(B)))
    out = np.stack([
        np.asarray(res.results[b]["out"]).astype(np.float32).reshape(Cout, H, W)
        for b in range(B)
    ])
    return out


# revision 12
# speedup vs baseline: 1.4025x; 1.0129x over previous
"""Self-contained Trainium2 (Bass/Tile) DeformConv2d kernel.

kernel(x, offset, weight) -> np.ndarray [B, Cout, H, W] float32.
Data-parallel over batch: one SPMD Bass program per NeuronCore (8 cores).

v2 design (vs v1 baseline):
- All bilinear weights / gather indices / gather table are built on the host
  (numpy) and passed as DRAM inputs; no on-device prep phases.
- Gather table is a bf16 "pair-row" layout: entry (y, x) holds image rows y
  and y+1 at column x concatenated (2C values). One 4C-value descriptor per
  sample covers all 4 bilinear neighbors (half the gather instructions and
  descriptor-gen of v1; DMA bytes unchanged).
- DVE does only the 4 per-sample weight multiplies (tensor_scalar, 4x mode).
- PE transpose-accumulates the 4 pieces into channel-major val (PSUM), then
  runs the per-tap GEMM.
- All PSUM->SBUF copies run on the otherwise-idle Activation engine.
- Output is written bf16 and upcast on the host.
"""
import sys
import numpy as np
import ml_dtypes

for _p in ("/opt/trn_rl_repo",):
    if _p not in sys.path:
        sys.path.insert(0, _p)

import concourse.bass as bass
import concourse.mybir as mybir
import concourse.tile as tile
from concourse import bacc
from concourse.masks import make_identity
from concourse.bass_utils import run_bass_kernel_spmd

f32 = mybir.dt.float32
bf16 = mybir.dt.bfloat16
i16 = mybir.dt.int16
Alu = mybir.AluOpType
P = 128
BF16 = ml_dtypes.bfloat16


def build_dcn(C=256, Cout=256, H=64, W=64, KH=3, KW=3, CHUNK_JT=8):
    HW = H * W
    S = HW // P              # 32 pixel slots of 128
    NT = KH * KW             # 9 taps
    CB = C // P              # 2 input-channel blocks
    MB = Cout // P           # 2 output-channel blocks
    assert S % CHUNK_JT == 0
    n_chunks = S // CHUNK_JT
    JC = CHUNK_JT * P        # 1024 pixels per chunk
    SWC = JC // 16           # idx columns per chunk (16-wrap layout)
    NNB = JC // 512          # moving-dim blocks for the GEMM
    TROWS = (H + 1) * W      # pair-table rows

    nc = bacc.Bacc("TRN2", target_bir_lowering=False, debug=False)

    # one extra zero row backs the overlapping pair view's last entry
    tbl = nc.declare_dram_parameter("tbl", [TROWS + 1, 2 * C], bf16,
                                    isOutput=False)
    idx = nc.declare_dram_parameter("idx", [P, NT, n_chunks * SWC], i16,
                                    isOutput=False)
    w4 = nc.declare_dram_parameter("w4", [P, NT, S, 4], f32, isOutput=False)
    wt = nc.declare_dram_parameter("wt", [P, NT, CB, Cout], bf16, isOutput=False)
    out = nc.declare_dram_parameter("out", [Cout, HW], bf16, isOutput=True)

    with tile.TileContext(nc) as tc:
        with tc.tile_pool(name="persist", bufs=1) as pp:
            wtb = pp.tile([P, NT, CB, Cout], bf16, name="wtb")
            w4t = pp.tile([P, NT, S, 4], f32, name="w4t")
            idxt = pp.tile([P, NT, n_chunks * SWC], i16, name="idxt")
            ident = pp.tile([P, P], bf16, name="ident")

            nc.sync.dma_start(out=idxt[:], in_=idx[:])
            nc.sync.dma_start(out=wtb[:], in_=wt[:])
            nc.sync.dma_start(out=w4t[:], in_=w4[:])
            make_identity(nc, ident[:])

            # overlapping-pair view: entry i covers table elements
            # [2C*i, 2C*i + 4C) -> one descriptor = 4 bilinear neighbors.
            tbl_pairs = bass.AP(tbl[:].tensor, 0, [[2 * C, TROWS], [1, 4 * C]])

            with (
                tc.tile_pool(name="gather", bufs=3) as g_pool,
                tc.tile_pool(name="prod", bufs=2) as pr_pool,
                tc.tile_pool(name="vout", bufs=2) as v_pool,
                tc.tile_pool(name="obuf", bufs=2) as o_pool,
                tc.tile_pool(name="psum_out", bufs=1, space="PSUM") as pso_pool,
                tc.tile_pool(name="psum_val", bufs=2, space="PSUM") as psv_pool,
            ):
                for ch in range(n_chunks):
                    out_ps = [
                        pso_pool.tile([P, JC], f32, space="PSUM", name=f"out_ps{m}")
                        for m in range(MB)
                    ]
                    for k in range(NT):
                        g = g_pool.tile([P, CHUNK_JT, 4 * C], bf16, name="g")
                        nc.gpsimd.dma_gather(
                            g[:], tbl_pairs,
                            idxt[:, k, ch * SWC:(ch + 1) * SWC],
                            JC, JC, 4 * C, elem_step=2 * C,
                        )
                        pr = pr_pool.tile([P, CHUNK_JT, 4, C], bf16, name="pr")
                        for jt in range(CHUNK_JT):
                            s_idx = ch * CHUNK_JT + jt
                            for n in range(4):
                                nc.vector.tensor_scalar(
                                    out=pr[:, jt, n, :],
                                    in0=g[:, jt, n * C:(n + 1) * C],
                                    scalar1=w4t[:, k, s_idx, n:n + 1],
                                    scalar2=None, op0=Alu.mult,
                                )
                        vsb = v_pool.tile([P, CB, JC], bf16, name="vsb")
                        for cb in range(CB):
                            # one cb in flight at a time: 2 PSUM val tiles
                            # (pool bufs=2) so PE never waits on the Act copy
                            val_ps = psv_pool.tile([P, JC], f32, space="PSUM",
                                                   name="val_ps")
                            for jt in range(CHUNK_JT):
                                for n in range(4):
                                    nc.tensor.matmul(
                                        out=val_ps[:, jt * P:(jt + 1) * P],
                                        lhsT=pr[:, jt, n, cb * P:(cb + 1) * P],
                                        rhs=ident[:],
                                        start=(n == 0), stop=(n == 3),
                                    )
                            nc.scalar.copy(out=vsb[:, cb, :], in_=val_ps[:])
                            for mb in range(MB):
                                for nb in range(NNB):
                                    nsl = slice(nb * 512, (nb + 1) * 512)
                                    nc.tensor.matmul(
                                        out=out_ps[mb][:, nsl],
                                        lhsT=wtb[:, k, cb, mb * P:(mb + 1) * P],
                                        rhs=vsb[:, cb, nsl],
                                        start=(k == 0 and cb == 0),
                                        stop=(k == NT - 1 and cb == CB - 1),
                                    )
                    for mb in range(MB):
                        ob = o_pool.tile([P, JC], bf16, name="ob")
                        nc.scalar.copy(out=ob[:], in_=out_ps[mb][:])
                        nc.sync.dma_start(
                            out=out[mb * P:(mb + 1) * P, ch * JC:(ch + 1) * JC],
                            in_=ob[:],
                        )

    nc.compile()
    return nc


def host_prep(x_b, offset_b, weight, H, W, KH, KW, PAD):
    """Per-core input map from one batch slice (numpy, f32)."""
    C = x_b.shape[0]
    Cout = weight.shape[0]
    HW = H * W
    S = HW // P
    NT = KH * KW
    CB = C // P

    # pair-row gather table: entry r=(y0+1)*W+x holds rows (y0, y0+1) at col x
    xt = x_b.reshape(C, H, W).transpose(1, 2, 0).astype(np.float32)  # [H, W, C]
    Z = np.zeros((H + 2, W, C), np.float32)
    Z[1:H + 1] = xt
    T = np.concatenate([Z[0:H + 1], Z[1:H + 2]], axis=-1)  # [(H+1), W, 2C]
    tbl = np.zeros(((H + 1) * W + 1, 2 * C), np.float32)
    tbl[:-1] = T.reshape((H + 1) * W, 2 * C)
    tbl = tbl.astype(BF16)

    # sample coords per (tap, pixel)
    off = offset_b.reshape(NT, 2, HW).astype(np.float32)
    j = np.arange(HW)
    ks = np.arange(NT)
    by = (j[None, :] // W - PAD + (ks // KW)[:, None]).astype(np.float32)
    bx = (j[None, :] % W - PAD + (ks % KW)[:, None]).astype(np.float32)
    py = by + off[:, 0]
    px = bx + off[:, 1]
    y0 = np.floor(py)
    x0 = np.floor(px)
    ly = (py - y0).astype(np.float32)
    lx = (px - x0).astype(np.float32)
    qy = np.clip(y0, -1, H - 1)
    sx = np.clip(x0, 0, W - 2)
    idx_lin = ((qy + 1) * W + sx).astype(np.int16)  # [NT, HW]

    wy0 = (1.0 - ly) * ((y0 >= 0) & (y0 <= H - 1))
    wyB = ly * ((y0 >= -1) & (y0 <= H - 2))
    vx0 = (x0 >= 0) & (x0 <= W - 1)
    vx1 = (x0 >= -1) & (x0 <= W - 2)
    wxA = (1.0 - lx) * vx0 * (x0 == sx) + lx * vx1 * ((x0 + 1) == sx)
    wxB = (1.0 - lx) * vx0 * (x0 == (sx + 1)) + lx * vx1 * ((x0 + 1) == (sx + 1))
    # piece order matches the gathered 4C row: [y0|x0, y1|x0, y0|x1, y1|x1]
    w4 = np.stack([wy0 * wxA, wyB * wxA, wy0 * wxB, wyB * wxB],
                  axis=-1).astype(np.float32)  # [NT, HW, 4]
    w4d = np.ascontiguousarray(
        w4.reshape(NT, S, P, 4).transpose(2, 0, 1, 3))  # [P, NT, S, 4]

    # 16-wrap idx layout: slice column c of chunk ch, partition q -> sample
    # i = c*16 + q (i = chunk-local pixel), replicated over 8 partition groups
    idxw = idx_lin.reshape(NT, HW // 16, 16).transpose(2, 0, 1)  # [16, NT, HW/16]
    idxw = np.ascontiguousarray(np.tile(idxw, (8, 1, 1))).astype(np.int16)

    wtd = np.ascontiguousarray(
        weight.reshape(Cout, CB, P, NT).transpose(2, 3, 1, 0)).astype(BF16)
    return {"tbl": tbl, "idx": idxw, "w4": w4d, "wt": wtd}


_NC_CACHE = {}


def _get_nc(key, **kw):
    if key not in _NC_CACHE:
        _NC_CACHE[key] = build_dcn(**kw)
    return _NC_CACHE[key]


def kernel(x, offset, weight):
    x = np.asarray(x, dtype=np.float32)
    offset = np.asarray(offset, dtype=np.float32)
    weight = np.asarray(weight, dtype=np.float32)
    B, C, H, W = x.shape
    Cout = weight.shape[0]
    KH, KW = weight.shape[2], weight.shape[3]
    assert B == 8 and C % 128 == 0 and Cout % 128 == 0
    nc = _get_nc((C, Cout, H, W, KH, KW), C=C, Cout=Cout, H=H, W=W,
                 KH=KH, KW=KW, CHUNK_JT=8)
    in_maps = [host_prep(x[b], offset[b], weight, H, W, KH, KW, 1)
               for b in range(B)]
    res = run_bass_kernel_spmd(nc, in_maps, list(range(B)))
    out = np.stack([
        np.asarray(res.results[b]["out"]).astype(np.float32).reshape(Cout, H, W)
        for b in range(B)
    ])
    return out


# revision 14
# speedup vs baseline: 1.4029x; 1.0003x over previous
"""Self-contained Trainium2 (Bass/Tile) DeformConv2d kernel.

kernel(x, offset, weight) -> np.ndarray [B, Cout, H, W] float32.
Data-parallel over batch: one SPMD Bass program per NeuronCore (8 cores).

v2 design (vs v1 baseline):
- All bilinear weights / gather indices / gather table are built on the host
  (numpy) and passed as DRAM inputs; no on-device prep phases.
- Gather table is a bf16 "pair-row" layout: entry (y, x) holds image rows y
  and y+1 at column x concatenated (2C values). One 4C-value descriptor per
  sample covers all 4 bilinear neighbors (half the gather instructions and
  descriptor-gen of v1; DMA bytes unchanged).
- DVE does only the 4 per-sample weight multiplies (tensor_scalar, 4x mode).
- PE transpose-accumulates the 4 pieces into channel-major val (PSUM), then
  runs the per-tap GEMM.
- All PSUM->SBUF copies run on the otherwise-idle Activation engine.
- Output is written bf16 and upcast on the host.
"""
import sys
import numpy as np
import ml_dtypes

for _p in ("/opt/trn_rl_repo",):
    if _p not in sys.path:
        sys.path.insert(0, _p)

import concourse.bass as bass
import concourse.mybir as mybir
import concourse.tile as tile
from concourse import bacc
from concourse.masks import make_identity
from concourse.bass_utils import run_bass_kernel_spmd

f32 = mybir.dt.float32
bf16 = mybir.dt.bfloat16
i16 = mybir.dt.int16
Alu = mybir.AluOpType
P = 128
BF16 = ml_dtypes.bfloat16


def build_dcn(C=256, Cout=256, H=64, W=64, KH=3, KW=3, CHUNK_JT=8):
    HW = H * W
    S = HW // P              # 32 pixel slots of 128
    NT = KH * KW             # 9 taps
    CB = C // P              # 2 input-channel blocks
    MB = Cout // P           # 2 output-channel blocks
    assert S % CHUNK_JT == 0
    n_chunks = S // CHUNK_JT
    JC = CHUNK_JT * P        # 1024 pixels per chunk
    SWC = JC // 16           # idx columns per chunk (16-wrap layout)
    NNB = JC // 512          # moving-dim blocks for the GEMM
    TROWS = (H + 1) * W      # pair-table rows

    nc = bacc.Bacc("TRN2", target_bir_lowering=False, debug=False)

    # one extra zero row backs the overlapping pair view's last entry
    tbl = nc.declare_dram_parameter("tbl", [TROWS + 1, 2 * C], bf16,
                                    isOutput=False)
    idx = nc.declare_dram_parameter("idx", [P, NT, n_chunks * SWC], i16,
                                    isOutput=False)
    w4 = nc.declare_dram_parameter("w4", [P, NT, S, 4], f32, isOutput=False)
    wt = nc.declare_dram_parameter("wt", [P, NT, CB, Cout], bf16, isOutput=False)
    out = nc.declare_dram_parameter("out", [Cout, HW], bf16, isOutput=True)

    with tile.TileContext(nc) as tc:
        with tc.tile_pool(name="persist", bufs=1) as pp:
            wtb = pp.tile([P, NT, CB, Cout], bf16, name="wtb")
            w4t = pp.tile([P, NT, S, 4], f32, name="w4t")
            idxt = pp.tile([P, NT, n_chunks * SWC], i16, name="idxt")
            ident = pp.tile([P, P], bf16, name="ident")

            nc.sync.dma_start(out=idxt[:], in_=idx[:])
            nc.sync.dma_start(out=wtb[:], in_=wt[:])
            nc.sync.dma_start(out=w4t[:], in_=w4[:])
            make_identity(nc, ident[:])

            # overlapping-pair view: entry i covers table elements
            # [2C*i, 2C*i + 4C) -> one descriptor = 4 bilinear neighbors.
            tbl_pairs = bass.AP(tbl[:].tensor, 0, [[2 * C, TROWS], [1, 4 * C]])

            with (
                tc.tile_pool(name="gather", bufs=3) as g_pool,
                tc.tile_pool(name="prod", bufs=16) as pr_pool,
                tc.tile_pool(name="vout", bufs=3) as v_pool,
                tc.tile_pool(name="obuf", bufs=2) as o_pool,
                tc.tile_pool(name="psum_out", bufs=1, space="PSUM") as pso_pool,
                tc.tile_pool(name="psum_val", bufs=2, space="PSUM") as psv_pool,
            ):
                for ch in range(n_chunks):
                    out_ps = [
                        pso_pool.tile([P, JC], f32, space="PSUM", name=f"out_ps{m}")
                        for m in range(MB)
                    ]
                    for k in range(NT):
                        g = g_pool.tile([P, CHUNK_JT, 4 * C], bf16, name="g")
                        nc.gpsimd.dma_gather(
                            g[:], tbl_pairs,
                            idxt[:, k, ch * SWC:(ch + 1) * SWC],
                            JC, JC, 4 * C, elem_step=2 * C,
                        )
                        # per-jt pr tiles: PE transposes stream right behind
                        # the DVE multiplies instead of waiting for all 32
                        prs = []
                        for jt in range(CHUNK_JT):
                            s_idx = ch * CHUNK_JT + jt
                            pr = pr_pool.tile([P, 4, C], bf16, name="pr")
                            prs.append(pr)
                            for n in range(4):
                                nc.vector.tensor_scalar(
                                    out=pr[:, n, :],
                                    in0=g[:, jt, n * C:(n + 1) * C],
                                    scalar1=w4t[:, k, s_idx, n:n + 1],
                                    scalar2=None, op0=Alu.mult,
                                )
                        vsb = v_pool.tile([P, CB, JC], bf16, name="vsb")
                        for cb in range(CB):
                            # one cb in flight at a time: 2 PSUM val tiles
                            # (pool bufs=2) so PE never waits on the Act copy
                            val_ps = psv_pool.tile([P, JC], f32, space="PSUM",
                                                   name="val_ps")
                            for jt in range(CHUNK_JT):
                                for n in range(4):
                                    nc.tensor.matmul(
                                        out=val_ps[:, jt * P:(jt + 1) * P],
                                        lhsT=prs[jt][:, n, cb * P:(cb + 1) * P],
                                        rhs=ident[:],
                                        start=(n == 0), stop=(n == 3),
                                    )
                            nc.scalar.copy(out=vsb[:, cb, :], in_=val_ps[:])
                            for mb in range(MB):
                                for nb in range(NNB):
                                    nsl = slice(nb * 512, (nb + 1) * 512)
                                    nc.tensor.matmul(
                                        out=out_ps[mb][:, nsl],
                                        lhsT=wtb[:, k, cb, mb * P:(mb + 1) * P],
                                        rhs=vsb[:, cb, nsl],
                                        start=(k == 0 and cb == 0),
                                        stop=(k == NT - 1 and cb == CB - 1),
                                    )
                    for mb in range(MB):
                        ob = o_pool.tile([P, JC], bf16, name="ob")
                        nc.scalar.copy(out=ob[:], in_=out_ps[mb][:])
                        nc.sync.dma_start(
                            out=out[mb * P:(mb + 1) * P, ch * JC:(ch + 1) * JC],
                            in_=ob[:],
                        )

    nc.compile()
    return nc


def host_prep(x_b, offset_b, weight, H, W, KH, KW, PAD):
    """Per-core input map from one batch slice (numpy, f32)."""
    C = x_b.shape[0]
    Cout = weight.shape[0]
    HW = H * W
    S = HW // P
    NT = KH * KW
    CB = C // P

    # pair-row gather table: entry r=(y0+1)*W+x holds rows (y0, y0+1) at col x
    xt = x_b.reshape(C, H, W).transpose(1, 2, 0).astype(np.float32)  # [H, W, C]
    Z = np.zeros((H + 2, W, C), np.float32)
    Z[1:H + 1] = xt
    T = np.concatenate([Z[0:H + 1], Z[1:H + 2]], axis=-1)  # [(H+1), W, 2C]
    tbl = np.zeros(((H + 1) * W + 1, 2 * C), np.float32)
    tbl[:-1] = T.reshape((H + 1) * W, 2 * C)
    tbl = tbl.astype(BF16)

    # sample coords per (tap, pixel)
    off = offset_b.reshape(NT, 2, HW).astype(np.float32)
    j = np.arange(HW)
    ks = np.arange(NT)
    by = (j[None, :] // W - PAD + (ks // KW)[:, None]).astype(np.float32)
    bx = (j[None, :] % W - PAD + (ks % KW)[:, None]).astype(np.float32)
    py = by + off[:, 0]
    px = bx + off[:, 1]
    y0 = np.floor(py)
    x0 = np.floor(px)
    ly = (py - y0).astype(np.float32)
    lx = (px - x0).astype(np.float32)
    qy = np.clip(y0, -1, H - 1)
    sx = np.clip(x0, 0, W - 2)
    idx_lin = ((qy + 1) * W + sx).astype(np.int16)  # [NT, HW]

    wy0 = (1.0 - ly) * ((y0 >= 0) & (y0 <= H - 1))
    wyB = ly * ((y0 >= -1) & (y0 <= H - 2))
    vx0 = (x0 >= 0) & (x0 <= W - 1)
    vx1 = (x0 >= -1) & (x0 <= W - 2)
    wxA = (1.0 - lx) * vx0 * (x0 == sx) + lx * vx1 * ((x0 + 1) == sx)
    wxB = (1.0 - lx) * vx0 * (x0 == (sx + 1)) + lx * vx1 * ((x0 + 1) == (sx + 1))
    # piece order matches the gathered 4C row: [y0|x0, y1|x0, y0|x1, y1|x1]
    w4 = np.stack([wy0 * wxA, wyB * wxA, wy0 * wxB, wyB * wxB],
                  axis=-1).astype(np.float32)  # [NT, HW, 4]
    w4d = np.ascontiguousarray(
        w4.reshape(NT, S, P, 4).transpose(2, 0, 1, 3))  # [P, NT, S, 4]

    # 16-wrap idx layout: slice column c of chunk ch, partition q -> sample
    # i = c*16 + q (i = chunk-local pixel), replicated over 8 partition groups
    idxw = idx_lin.reshape(NT, HW // 16, 16).transpose(2, 0, 1)  # [16, NT, HW/16]
    idxw = np.ascontiguousarray(np.tile(idxw, (8, 1, 1))).astype(np.int16)

    wtd = np.ascontiguousarray(
        weight.reshape(Cout, CB, P, NT).transpose(2, 3, 1, 0)).astype(BF16)
    return {"tbl": tbl, "idx": idxw, "w4": w4d, "wt": wtd}


_NC_CACHE = {}


def _get_nc(key, **kw):
    if key not in _NC_CACHE:
        _NC_CACHE[key] = build_dcn(**kw)
    return _NC_CACHE[key]


def kernel(x, offset, weight):
    x = np.asarray(x, dtype=np.float32)
    offset = np.asarray(offset, dtype=np.float32)
    weight = np.asarray(weight, dtype=np.float32)
    B, C, H, W = x.shape
    Cout = weight.shape[0]
    KH, KW = weight.shape[2], weight.shape[3]
    assert B == 8 and C % 128 == 0 and Cout % 128 == 0
    nc = _get_nc((C, Cout, H, W, KH, KW), C=C, Cout=Cout, H=H, W=W,
                 KH=KH, KW=KW, CHUNK_JT=8)
    in_maps = [host_prep(x[b], offset[b], weight, H, W, KH, KW, 1)
               for b in range(B)]
    res = run_bass_kernel_spmd(nc, in_maps, list(range(B)))
    out = np.stack([
        np.asarray(res.results[b]["out"]).astype(np.float32).reshape(Cout, H, W)
        for b in range(B)
    ])
    return out
